# revision 15
# baseline (speedup 1.0000x reference)
"""Sliding-window attention Trainium2 Bass kernel.

Problem: B=4, H=32, L=4096, D=128, window=512.
reference: attends over the LAST w=512 key/value positions; query row i may
only see window slot j when j <= i.

Sharding: B*H = 128 (b,h) pairs split across 8 cores -> 16 heads/core.
Pure data parallelism, no collectives.

Per-group (512 queries) on-device algorithm, all data bf16:
  S^T halves [128, 2c x 512q] = (K^T chunk)^T . (Q^T group)    (PE -> s psum)
  mask-add diagonal blocks on the first 512 queries            (DVE)
  P^T = exp(S^T / sqrt(D)), two [128,1024] instrs              (ACT, 2/group)
  PV+rowsum fused, c-outer: per 128-query block qb
      O[q, 0:129] += P_blk(c,qb)^T @ [V_c | ones]              (PE -> o psum)
    col 128 of each block is the softmax denominator; O arrives in [q, d]
    layout with per-query sums on the partition axis.
  recip[128,4] = approx 1/sums (one strided DVE op)            (DVE)
  og = O * recip (one 3D broadcast tensor_mul)                 (DVE -> SBUF)

PSUM: s pool 2x[128,1024] (4 banks, freed by ACT - short S->ACT chain) and a
DETACHED o pool 2x[128,1024] (4 banks). O block qb sits at f32 col offset
{0,130,512,642} of its o tile: each [128,129] accumulation region is inside
one bank and 8B-aligned. Keeping O out of the s tiles is what lets the PV
matmuls and the DVE drain run decoupled from the S->ACT chain (matmul
start=True is dependency-modeled as a whole-tile write, so any same-tile DVE
reader would serialize the PE).

The emission is software-pipelined one group deep: PE sees
[S-matmuls(g); PV(g-1)] and never waits on the ACT exp. Expected bound:
ACT busy ~2.3us/group.

Host pre-transposes Q/K to [head, D, L] bf16, bakes the ones column into V,
and the output returns as [head, g, q_local, qb, d] needing only a cheap
numpy transpose.
"""

import math
from contextlib import ExitStack

import numpy as np

N_CORES = 8
B, H, L, D = 4, 32, 4096, 128
W = 512            # window
HEADS_PER_CORE = (B * H) // N_CORES   # 16
QG = 512           # queries per group
NG = L // QG       # groups per head (8)
NCHUNK = W // 128  # 4 window chunks
SCALE = 1.0 / math.sqrt(D)
# f32 column offset of O block qb inside its o psum tile (bank-contained,
# 8-byte aligned)
O_OFF = (0, 130, 512, 642)

_COMPILED = None


def _build():
    import concourse.tile as tile
    from concourse import bacc, mybir

    nc = bacc.Bacc("TRN2", target_bir_lowering=False, debug=False,
                   num_devices=N_CORES)

    bf16 = mybir.dt.bfloat16
    f32 = mybir.dt.float32

    qT = nc.dram_tensor("qT", [HEADS_PER_CORE, D, L], bf16, kind="ExternalInput").ap()
    kT = nc.dram_tensor("kT", [HEADS_PER_CORE, D, W], bf16, kind="ExternalInput").ap()
    # [head, slot_within_chunk, chunk*(D+1)]: V chunk columns + a ones column
    vaug = nc.dram_tensor("vaug", [HEADS_PER_CORE, 128, NCHUNK * (D + 1)], bf16,
                          kind="ExternalInput").ap()
    # 0/1 lower triangle: tri[j, i] = 1 where i >= j (query may see slot)
    triT = nc.dram_tensor("triT", [128, 128], bf16, kind="ExternalInput").ap()
    outT = nc.dram_tensor("outT", [HEADS_PER_CORE, NG, 128, QG], bf16,
                          kind="ExternalOutput").ap()

    with tile.TileContext(nc) as tc:
        with ExitStack() as ctx:
            const = ctx.enter_context(tc.tile_pool(name="const", bufs=1))
            kt_pool = ctx.enter_context(tc.tile_pool(name="kt", bufs=2))
            v_pool = ctx.enter_context(tc.tile_pool(name="v", bufs=2))
            q_pool = ctx.enter_context(tc.tile_pool(name="q", bufs=2))
            p_pool = ctx.enter_context(tc.tile_pool(name="p", bufs=3))
            og_pool = ctx.enter_context(tc.tile_pool(name="og", bufs=3))
            rc_pool = ctx.enter_context(tc.tile_pool(name="rc", bufs=3))
            s_psum = ctx.enter_context(tc.tile_pool(name="s_ps", bufs=2, space="PSUM"))
            o_psum = ctx.enter_context(tc.tile_pool(name="o_ps", bufs=2, space="PSUM"))

            tri_t = const.tile([128, 128], bf16, tag="tri")
            nc.gpsimd.dma_start(tri_t[:], triT[:])
            # prewarm the exp table set so the one-time ~2.7us ACT_TABLE_LOAD
            # overlaps the first head's DMA instead of stalling group 0
            warm_t = const.tile([1, 1], f32, tag="warm")
            nc.scalar.activation(warm_t[:], tri_t[0:1, 0:1],
                                 mybir.ActivationFunctionType.Exp, scale=1.0)

            head_tiles = {}

            def load_head(h, first=False):
                kt_t = kt_pool.tile([128, W], bf16, tag="kt")
                nc.sync.dma_start(kt_t[:], kT[h])
                v_t = v_pool.tile([128, NCHUNK * (D + 1)], bf16, tag="v")
                nc.sync.dma_start(v_t[:], vaug[h])
                qt_t = q_pool.tile([128, L], bf16, tag="q")
                if first:
                    # split so group 0's query slice lands ~3us earlier and
                    # the first S matmuls aren't gated on the full 1MB load
                    nc.sync.dma_start(qt_t[:, 0:QG], qT[h, :, 0:QG])
                    nc.sync.dma_start(qt_t[:, QG:], qT[h, :, QG:])
                else:
                    nc.sync.dma_start(qt_t[:], qT[h])
                head_tiles[h] = (kt_t, v_t, qt_t)

            def emit_front(h, g):
                """S matmuls + mask + exp for group (h, g). Returns stage."""
                kt_t, v_t, qt_t = head_tiles[h]
                p_t = p_pool.tile([128, NCHUNK * QG], bf16, tag="p")
                for half in range(2):
                    s_t = s_psum.tile([128, 2 * QG], f32, tag="s")
                    for ci in range(2):
                        c = 2 * half + ci
                        # Full matmul even at g==0 (masked blocks are never
                        # read by PV): keeps every psum byte freshly written,
                        # so the exp below never reads stale data.
                        nc.tensor.matmul(
                            s_t[:, ci * QG:(ci + 1) * QG],
                            lhsT=kt_t[:, c * 128:(c + 1) * 128],
                            rhs=qt_t[:, g * QG:(g + 1) * QG],
                            start=True, stop=True,
                        )
                    nc.scalar.activation(
                        p_t[:, half * 2 * QG:(half + 1) * 2 * QG], s_t[:],
                        mybir.ActivationFunctionType.Exp, scale=SCALE)
                return (h, g, p_t)

            def emit_back(stage):
                """Fused PV+rowsum, recip, normalize, store for a stage."""
                h, g, p_t = stage
                kt_t, v_t, qt_t = head_tiles[h]
                o_t = o_psum.tile([128, 2 * QG], f32, tag="o")
                og_t = og_pool.tile([128, QG], bf16, tag="og")
                rc_t = rc_pool.tile([128, NCHUNK], f32, tag="rc")
                if g == 0:
                    # group 0 causal mask, applied POST-exp (so it never sits
                    # between the S matmuls and the ACT instr) and emitted in
                    # the back phase (so it never delays the previous groups'
                    # drains in the DVE queue): zero the diagonal P block of
                    # each chunk. Fully-masked blocks are skipped by the PV
                    # matmuls; the ones column sums the zeroed P, keeping the
                    # denominators exact.
                    for c in range(NCHUNK):
                        blk = slice(c * QG + c * 128, c * QG + (c + 1) * 128)
                        nc.vector.tensor_mul(p_t[:, blk], p_t[:, blk], tri_t[:])
                # qb-outer: each [128,129] dst accumulates c=0..last before
                # the next group opens -- two accumulation groups must never
                # be concurrently open in the same PSUM bank (start=True
                # clears has_written bank-wide), and qb0/qb1 share bank 0.
                for qb in range(NCHUNK):
                    last_c = NCHUNK - 1 if g > 0 else qb
                    for c in range(last_c + 1):
                        nc.tensor.matmul(
                            o_t[:, O_OFF[qb]:O_OFF[qb] + D + 1],
                            lhsT=p_t[:, c * QG + qb * 128:c * QG + (qb + 1) * 128],
                            rhs=v_t[:, c * (D + 1):(c + 1) * (D + 1)],
                            start=(c == 0), stop=(c == last_c),
                        )
                # sums live at col O_OFF[qb]+D: offsets {128,258,640,770} =
                # 128 + a*512 + b*130 -- one strided reciprocal
                half_view = o_t.rearrange("p (a x) -> p a x", a=2)   # [128,2,512]
                sums_ap = half_view[:, :, D:D + 131:130]             # [128,2,2]
                rc_view = rc_t.rearrange("p (a b) -> p a b", a=2)
                nc.vector.reciprocal_approx_fast(rc_view, sums_ap)
                # O block view [128,2,2,128] with strides (512,130,1)
                o_blocks = half_view[:, :, 0:260].rearrange(
                    "p a (b x) -> p a b x", b=2)[:, :, :, 0:D]
                og_view = og_t.rearrange("p (a b x) -> p a b x", a=2, b=2)
                rc_bcast = rc_view.unsqueeze(3).broadcast_to([128, 2, 2, D])
                # normalize: og[qb, d] = O[qb, d] * rc[qb], one tensor_mul
                nc.vector.tensor_mul(og_view, o_blocks, rc_bcast)
                nc.sync.dma_start(outT[h, g], og_t[:])
                if g == NG - 1:
                    del head_tiles[h]

            prev = None
            load_head(0, first=True)
            for it in range(HEADS_PER_CORE * NG):
                h, g = divmod(it, NG)
                if g == NG // 2 and h + 1 < HEADS_PER_CORE:
                    load_head(h + 1)   # prefetch next head during this one
                cur = emit_front(h, g)
                if prev is not None:
                    emit_back(prev)
                prev = cur
            emit_back(prev)

    nc.compile()
    return nc


def _get_compiled():
    global _COMPILED
    if _COMPILED is None:
        _COMPILED = _build()
    return _COMPILED


def _make_in_maps(query, keys, values):
    import ml_dtypes
    bf16 = ml_dtypes.bfloat16

    q = np.asarray(query, dtype=np.float32)
    k = np.asarray(keys, dtype=np.float32)
    v = np.asarray(values, dtype=np.float32)

    qf = q.reshape(B * H, L, D)
    kf = k.reshape(B * H, L, D)[:, L - W:, :]
    vf = v.reshape(B * H, L, D)[:, L - W:, :]

    qT = np.ascontiguousarray(qf.transpose(0, 2, 1)).astype(bf16)
    kT = np.ascontiguousarray(kf.transpose(0, 2, 1)).astype(bf16)
    vc = vf.reshape(B * H, NCHUNK, 128, D).transpose(0, 2, 1, 3)
    vaug = np.ones((B * H, 128, NCHUNK, D + 1), dtype=np.float32)
    vaug[:, :, :, :D] = vc
    vaug = vaug.reshape(B * H, 128, NCHUNK * (D + 1)).astype(bf16)

    tri = (np.arange(128)[None, :] >= np.arange(128)[:, None]).astype(bf16)

    in_maps = []
    for core in range(N_CORES):
        s = slice(core * HEADS_PER_CORE, (core + 1) * HEADS_PER_CORE)
        in_maps.append({
            "qT": np.ascontiguousarray(qT[s]),
            "kT": np.ascontiguousarray(kT[s]),
            "vaug": np.ascontiguousarray(vaug[s]),
            "triT": tri,
        })
    return in_maps


def kernel(query, keys, values, window_size):
    from concourse.bass_utils import run_bass_kernel_spmd

    w = int(window_size)
    assert np.asarray(query).shape == (B, H, L, D) and w == W

    nc = _get_compiled()
    in_maps = _make_in_maps(query, keys, values)
    res = run_bass_kernel_spmd(nc, in_maps, core_ids=list(range(N_CORES)))
    outs = []
    for c in range(N_CORES):
        o = np.asarray(res.results[c]["outT"]).astype(np.float32)
        o = o.reshape(HEADS_PER_CORE, NG, 128, NCHUNK, D).transpose(0, 1, 3, 2, 4)
        outs.append(o.reshape(HEADS_PER_CORE, L, D))
    return np.concatenate(outs, axis=0).reshape(B, H, L, D)


# revision 17
# speedup vs baseline: 1.0095x; 1.0095x over previous
"""Sliding-window attention Trainium2 Bass kernel.

Problem: B=4, H=32, L=4096, D=128, window=512.
reference: attends over the LAST w=512 key/value positions; query row i may
only see window slot j when j <= i.

Sharding: B*H = 128 (b,h) pairs split across 8 cores -> 16 heads/core.
Pure data parallelism, no collectives.

Per-group (512 queries) on-device algorithm, all data bf16:
  S^T halves [128, 2c x 512q] = (K^T chunk)^T . (Q^T group)    (PE -> s psum)
  mask-add diagonal blocks on the first 512 queries            (DVE)
  P^T = exp(S^T / sqrt(D)), two [128,1024] instrs              (ACT, 2/group)
  PV+rowsum fused, c-outer: per 128-query block qb
      O[q, 0:129] += P_blk(c,qb)^T @ [V_c | ones]              (PE -> o psum)
    col 128 of each block is the softmax denominator; O arrives in [q, d]
    layout with per-query sums on the partition axis.
  recip[128,4] = approx 1/sums (one strided DVE op)            (DVE)
  og = O * recip (one 3D broadcast tensor_mul)                 (DVE -> SBUF)

PSUM: s pool 2x[128,1024] (4 banks, freed by ACT - short S->ACT chain) and a
DETACHED o pool 2x[128,1024] (4 banks). O block qb sits at f32 col offset
{0,130,512,642} of its o tile: each [128,129] accumulation region is inside
one bank and 8B-aligned. Keeping O out of the s tiles is what lets the PV
matmuls and the DVE drain run decoupled from the S->ACT chain (matmul
start=True is dependency-modeled as a whole-tile write, so any same-tile DVE
reader would serialize the PE).

The emission is software-pipelined one group deep: PE sees
[S-matmuls(g); PV(g-1)] and never waits on the ACT exp. Expected bound:
ACT busy ~2.3us/group.

Host pre-transposes Q/K to [head, D, L] bf16, bakes the ones column into V,
and the output returns as [head, g, q_local, qb, d] needing only a cheap
numpy transpose.
"""

import math
from contextlib import ExitStack

import numpy as np

N_CORES = 8
B, H, L, D = 4, 32, 4096, 128
W = 512            # window
HEADS_PER_CORE = (B * H) // N_CORES   # 16
QG = 512           # queries per group
NG = L // QG       # groups per head (8)
NCHUNK = W // 128  # 4 window chunks
SCALE = 1.0 / math.sqrt(D)
# f32 column offset of O block qb inside its o psum tile (bank-contained,
# 8-byte aligned)
O_OFF = (0, 130, 512, 642)

_COMPILED = None


def _build():
    import concourse.tile as tile
    from concourse import bacc, mybir

    nc = bacc.Bacc("TRN2", target_bir_lowering=False, debug=False,
                   num_devices=N_CORES)

    bf16 = mybir.dt.bfloat16
    f32 = mybir.dt.float32

    qT = nc.dram_tensor("qT", [HEADS_PER_CORE, D, L], bf16, kind="ExternalInput").ap()
    kT = nc.dram_tensor("kT", [HEADS_PER_CORE, D, W], bf16, kind="ExternalInput").ap()
    # [head, slot_within_chunk, chunk*(D+1)]: V chunk columns + a ones column
    vaug = nc.dram_tensor("vaug", [HEADS_PER_CORE, 128, NCHUNK * (D + 1)], bf16,
                          kind="ExternalInput").ap()
    # 0/1 lower triangle: tri[j, i] = 1 where i >= j (query may see slot)
    triT = nc.dram_tensor("triT", [128, 128], bf16, kind="ExternalInput").ap()
    outT = nc.dram_tensor("outT", [HEADS_PER_CORE, NG, 128, QG], bf16,
                          kind="ExternalOutput").ap()

    with tile.TileContext(nc) as tc:
        with ExitStack() as ctx:
            const = ctx.enter_context(tc.tile_pool(name="const", bufs=1))
            kt_pool = ctx.enter_context(tc.tile_pool(name="kt", bufs=2))
            v_pool = ctx.enter_context(tc.tile_pool(name="v", bufs=2))
            q_pool = ctx.enter_context(tc.tile_pool(name="q", bufs=2))
            p_pool = ctx.enter_context(tc.tile_pool(name="p", bufs=3))
            og_pool = ctx.enter_context(tc.tile_pool(name="og", bufs=3))
            rc_pool = ctx.enter_context(tc.tile_pool(name="rc", bufs=3))
            s_psum = ctx.enter_context(tc.tile_pool(name="s_ps", bufs=2, space="PSUM"))
            o_psum = ctx.enter_context(tc.tile_pool(name="o_ps", bufs=2, space="PSUM"))

            tri_t = const.tile([128, 128], bf16, tag="tri")
            nc.gpsimd.dma_start(tri_t[:], triT[:])
            # prewarm the exp table set so the one-time ~2.7us ACT_TABLE_LOAD
            # overlaps the first head's DMA instead of stalling group 0
            warm_t = const.tile([1, 1], f32, tag="warm")
            nc.scalar.activation(warm_t[:], tri_t[0:1, 0:1],
                                 mybir.ActivationFunctionType.Exp, scale=1.0)

            head_tiles = {}

            def load_head(h, first=False):
                kt_t = kt_pool.tile([128, W], bf16, tag="kt")
                nc.sync.dma_start(kt_t[:], kT[h])
                v_t = v_pool.tile([128, NCHUNK * (D + 1)], bf16, tag="v")
                nc.sync.dma_start(v_t[:], vaug[h])
                qt_t = q_pool.tile([128, L], bf16, tag="q")
                if first:
                    # split so group 0's query slice lands ~3us earlier and
                    # the first S matmuls aren't gated on the full 1MB load
                    nc.sync.dma_start(qt_t[:, 0:QG], qT[h, :, 0:QG])
                    nc.sync.dma_start(qt_t[:, QG:], qT[h, :, QG:])
                else:
                    nc.sync.dma_start(qt_t[:], qT[h])
                head_tiles[h] = (kt_t, v_t, qt_t)

            def emit_front(h, g):
                """S matmuls + mask + exp for group (h, g). Returns stage."""
                kt_t, v_t, qt_t = head_tiles[h]
                p_t = p_pool.tile([128, NCHUNK * QG], bf16, tag="p")
                for half in range(2):
                    s_t = s_psum.tile([128, 2 * QG], f32, tag="s")
                    for ci in range(2):
                        c = 2 * half + ci
                        # Full matmul even at g==0 (masked blocks are never
                        # read by PV): keeps every psum byte freshly written,
                        # so the exp below never reads stale data.
                        nc.tensor.matmul(
                            s_t[:, ci * QG:(ci + 1) * QG],
                            lhsT=kt_t[:, c * 128:(c + 1) * 128],
                            rhs=qt_t[:, g * QG:(g + 1) * QG],
                            start=True, stop=True,
                        )
                    nc.scalar.activation(
                        p_t[:, half * 2 * QG:(half + 1) * 2 * QG], s_t[:],
                        mybir.ActivationFunctionType.Exp, scale=SCALE)
                if g == 0:
                    # group 0 causal mask, applied POST-exp so it never sits
                    # between the S matmuls and the ACT instr: zero the
                    # diagonal P block of each chunk a full pipeline stage
                    # before the PV matmuls read it. Fully-masked blocks are
                    # skipped by the PV matmuls; the ones column sums the
                    # zeroed P, keeping the denominators exact.
                    for c in range(NCHUNK):
                        blk = slice(c * QG + c * 128, c * QG + (c + 1) * 128)
                        nc.vector.tensor_mul(p_t[:, blk], p_t[:, blk], tri_t[:])
                return (h, g, p_t)

            def emit_back(stage):
                """Fused PV+rowsum, recip, normalize, store for a stage."""
                h, g, p_t = stage
                kt_t, v_t, qt_t = head_tiles[h]
                o_t = o_psum.tile([128, 2 * QG], f32, tag="o")
                og_t = og_pool.tile([128, QG], bf16, tag="og")
                rc_t = rc_pool.tile([128, NCHUNK], f32, tag="rc")

                # qb-outer: each [128,129] dst accumulates c=0..last before
                # the next group opens -- two accumulation groups must never
                # be concurrently open in the same PSUM bank (start=True
                # clears has_written bank-wide), and qb0/qb1 share bank 0.
                for qb in range(NCHUNK):
                    last_c = NCHUNK - 1 if g > 0 else qb
                    for c in range(last_c + 1):
                        nc.tensor.matmul(
                            o_t[:, O_OFF[qb]:O_OFF[qb] + D + 1],
                            lhsT=p_t[:, c * QG + qb * 128:c * QG + (qb + 1) * 128],
                            rhs=v_t[:, c * (D + 1):(c + 1) * (D + 1)],
                            start=(c == 0), stop=(c == last_c),
                        )
                # sums live at col O_OFF[qb]+D: offsets {128,258,640,770} =
                # 128 + a*512 + b*130 -- one strided reciprocal
                half_view = o_t.rearrange("p (a x) -> p a x", a=2)   # [128,2,512]
                sums_ap = half_view[:, :, D:D + 131:130]             # [128,2,2]
                rc_view = rc_t.rearrange("p (a b) -> p a b", a=2)
                nc.vector.reciprocal_approx_fast(rc_view, sums_ap)
                # O block view [128,2,2,128] with strides (512,130,1)
                o_blocks = half_view[:, :, 0:260].rearrange(
                    "p a (b x) -> p a b x", b=2)[:, :, :, 0:D]
                og_view = og_t.rearrange("p (a b x) -> p a b x", a=2, b=2)
                rc_bcast = rc_view.unsqueeze(3).broadcast_to([128, 2, 2, D])
                # normalize: og[qb, d] = O[qb, d] * rc[qb], one tensor_mul
                nc.vector.tensor_mul(og_view, o_blocks, rc_bcast)
                nc.sync.dma_start(outT[h, g], og_t[:])
                if g == NG - 1:
                    del head_tiles[h]

            prev = None
            load_head(0, first=True)
            for it in range(HEADS_PER_CORE * NG):
                h, g = divmod(it, NG)
                if g == NG // 2 and h + 1 < HEADS_PER_CORE:
                    load_head(h + 1)   # prefetch next head during this one
                cur = emit_front(h, g)
                if prev is not None:
                    emit_back(prev)
                prev = cur
            emit_back(prev)

    nc.compile()
    return nc


def _get_compiled():
    global _COMPILED
    if _COMPILED is None:
        _COMPILED = _build()
    return _COMPILED


def _make_in_maps(query, keys, values):
    import ml_dtypes
    bf16 = ml_dtypes.bfloat16

    q = np.asarray(query, dtype=np.float32)
    k = np.asarray(keys, dtype=np.float32)
    v = np.asarray(values, dtype=np.float32)

    qf = q.reshape(B * H, L, D)
    kf = k.reshape(B * H, L, D)[:, L - W:, :]
    vf = v.reshape(B * H, L, D)[:, L - W:, :]

    qT = np.ascontiguousarray(qf.transpose(0, 2, 1)).astype(bf16)
    kT = np.ascontiguousarray(kf.transpose(0, 2, 1)).astype(bf16)
    vc = vf.reshape(B * H, NCHUNK, 128, D).transpose(0, 2, 1, 3)
    vaug = np.ones((B * H, 128, NCHUNK, D + 1), dtype=np.float32)
    vaug[:, :, :, :D] = vc
    vaug = vaug.reshape(B * H, 128, NCHUNK * (D + 1)).astype(bf16)

    tri = (np.arange(128)[None, :] >= np.arange(128)[:, None]).astype(bf16)

    in_maps = []
    for core in range(N_CORES):
        s = slice(core * HEADS_PER_CORE, (core + 1) * HEADS_PER_CORE)
        in_maps.append({
            "qT": np.ascontiguousarray(qT[s]),
            "kT": np.ascontiguousarray(kT[s]),
            "vaug": np.ascontiguousarray(vaug[s]),
            "triT": tri,
        })
    return in_maps


def kernel(query, keys, values, window_size):
    from concourse.bass_utils import run_bass_kernel_spmd

    w = int(window_size)
    assert np.asarray(query).shape == (B, H, L, D) and w == W

    nc = _get_compiled()
    in_maps = _make_in_maps(query, keys, values)
    res = run_bass_kernel_spmd(nc, in_maps, core_ids=list(range(N_CORES)))
    outs = []
    for c in range(N_CORES):
        o = np.asarray(res.results[c]["outT"]).astype(np.float32)
        o = o.reshape(HEADS_PER_CORE, NG, 128, NCHUNK, D).transpose(0, 1, 3, 2, 4)
        outs.append(o.reshape(HEADS_PER_CORE, L, D))
    return np.concatenate(outs, axis=0).reshape(B, H, L, D)
